# revision 1
# baseline (speedup 1.0000x reference)
"""BCM_Conv2d_fft kernel for Trainium2 (8 NeuronCores, batch-parallel).

The reference is a block-circulant 3x3 conv computed via per-block
rfft/irfft over the channel-block axis (block size 8). Per-frequency the
block products are independent, so in a real-DFT channel basis the
256->256 channel mixing matrix of each conv tap is block-diagonal with
frequency groups {f0:32, f4:32, f1:64, f2:64, f3:64}. Grouping
{f0,f4,f1} -> chunk0 and {f2,f3} -> chunk1 makes every tap's mixing
matrix chunk-diagonal: the conv needs 9 matmuls per output tile per
chunk instead of 18 - half the direct-conv PE work.

Device pipeline per core (one image):
  1. fwd:  xhat = A @ x       per pixel (A = real-DFT, freq-major rows)
  2. conv: ohat = sum_pos M_pos @ shift(xhat)   (chunk-diagonal M)
  3. inv:  out  = Ainv @ ohat + b

A, M_pos, Ainv are built on host from the tiny weight w [32,288,8] and
shipped as float32r stationary blocks. Matmuls run in float32r (full PE
rate at N>=256). Sharding: batch B=8 -> one image per core.
"""

import os

import numpy as np

import concourse.bacc as bacc
import concourse.mybir as mybir
import concourse.tile as tile
from concourse.bass import ts
from concourse.bass_utils import run_bass_kernel_spmd

N_CORES = 8
C = 256
H = W = 56
HP = H + 2
KK = 3
BS = 8
L = H * W
RPT = 8                  # output rows per tile
NT = RPT * W             # 448 pixels per tile
N_TILES = L // NT        # 7
MCH = C // 128           # 2 channel chunks

F32 = mybir.dt.float32
F32R = mybir.dt.float32r

# weight block column indices in the packed wts tensor [128, 26*128]
FWD_BLK = lambda i, c: i * MCH + c            # i = in chunk, c = out chunk
CONV_BLK = lambda pos, c: 4 + pos * MCH + c
INV_BLK = lambda k, m: 22 + k * MCH + m
N_BLKS = 26

LAST_RESULT = None


def _freq_matrices(w: np.ndarray):
    """Build A [256,256], Ms (9x [256,256] chunk-diag), Ainv from w."""
    F = np.zeros((8, 8))
    FI = np.fft.rfft(np.eye(8), axis=-1)
    F[0] = FI[:, 0].real
    F[1], F[2] = FI[:, 1].real, FI[:, 1].imag
    F[3], F[4] = FI[:, 2].real, FI[:, 2].imag
    F[5], F[6] = FI[:, 3].real, FI[:, 3].imag
    F[7] = FI[:, 4].real

    def fm(bk, comp):
        if comp == 0:
            return bk
        if comp == 7:
            return 32 + bk
        if comp in (1, 2):
            return 64 + 2 * bk + (comp - 1)
        if comp in (3, 4):
            return 128 + 2 * bk + (comp - 3)
        return 192 + 2 * bk + (comp - 5)

    A = np.zeros((256, 256))
    for bk in range(32):
        for comp in range(8):
            A[fm(bk, comp), bk * 8:(bk + 1) * 8] = F[comp]
    Ainv = np.linalg.inv(A)

    wf = np.fft.rfft(w.astype(np.float64), axis=-1)  # [32, 288, 5]
    Ms = []
    for pos in range(9):
        M = np.zeros((256, 256))
        for pb in range(32):
            for kb in range(32):
                kc = pos * 32 + kb
                M[fm(pb, 0), fm(kb, 0)] += wf[pb, kc, 0].real
                M[fm(pb, 7), fm(kb, 7)] += wf[pb, kc, 4].real
                for fi in range(3):
                    re_i, im_i = 1 + 2 * fi, 2 + 2 * fi
                    Wr, Wi = wf[pb, kc, fi + 1].real, wf[pb, kc, fi + 1].imag
                    M[fm(pb, re_i), fm(kb, re_i)] += Wr
                    M[fm(pb, re_i), fm(kb, im_i)] += -Wi
                    M[fm(pb, im_i), fm(kb, re_i)] += Wi
                    M[fm(pb, im_i), fm(kb, im_i)] += Wr
        Ms.append(M)
    return A, Ms, Ainv


def _pack_weights(w: np.ndarray) -> np.ndarray:
    """-> [128, 26*128] float32: lhsT blocks for fwd, conv, inv stages."""
    A, Ms, Ainv = _freq_matrices(w)
    wts = np.zeros((128, N_BLKS * 128), np.float32)

    def put(idx, mat):  # mat [K=128, M=128] already transposed for lhsT
        wts[:, idx * 128:(idx + 1) * 128] = mat.astype(np.float32)

    sl = lambda i: slice(i * 128, (i + 1) * 128)
    for i in range(MCH):
        for c in range(MCH):
            put(FWD_BLK(i, c), A[sl(c), sl(i)].T)
    for pos in range(9):
        for c in range(MCH):
            put(CONV_BLK(pos, c), Ms[pos][sl(c), sl(c)].T)
    for k in range(MCH):
        for m in range(MCH):
            put(INV_BLK(k, m), Ainv[sl(m), sl(k)].T)
    return wts


def _kernel_body(tc, x, wts, bias, out):
    nc = tc.nc
    with (
        tc.tile_pool(name="const", bufs=1) as const_pool,
        tc.tile_pool(name="xp", bufs=1) as xp_pool,
        tc.tile_pool(name="xh", bufs=1) as xh_pool,
        tc.tile_pool(name="oh", bufs=6) as oh_pool,
        tc.tile_pool(name="ob", bufs=4) as ob_pool,
        tc.tile_pool(name="psf", bufs=3, space="PSUM") as psf_pool,
        tc.tile_pool(name="psc", bufs=3, space="PSUM") as psc_pool,
        tc.tile_pool(name="psi", bufs=2, space="PSUM") as psi_pool,
    ):
        # DMA issue order is first-needed-first: fwd weights, then the
        # first input rows, then the rest. A [128, N] dma_start fans out
        # across DMA engines on its own, so keep transfers big; x chunk 1
        # is issued from gpsimd so the two chunks' issues run in parallel.
        wt_sb = const_pool.tile([128, N_BLKS * 128], F32R)
        blk = lambda idx: wt_sb[:, ts(idx, 128)]
        nc.sync.dma_start(out=wt_sb[:, 0:4 * 128], in_=wts[:, 0:4 * 128])
        # conv blocks from the (otherwise idle) ACT issuer so their
        # transfers overlap the x pieces on sync/gpsimd queues
        nc.scalar.dma_start(out=wt_sb[:, 4 * 128:13 * 128],
                            in_=wts[:, 4 * 128:13 * 128])
        nc.scalar.dma_start(out=wt_sb[:, 13 * 128:22 * 128],
                            in_=wts[:, 13 * 128:22 * 128])

        xq = []
        for i in range(MCH):
            xq_t = xp_pool.tile([128, HP * HP], F32R, tag=f"xp{i}")
            xq.append(xq_t)
        row_splits = [0, 8, 16, 34, HP]
        eng = [nc.sync, nc.gpsimd]
        for r0, r1 in zip(row_splits[:-1], row_splits[1:]):
            for i in range(MCH):
                eng[i].dma_start(
                    out=xq[i][:, r0 * HP:r1 * HP],
                    in_=x[ts(i, 128), r0:r1, :].rearrange("p h w -> p (h w)"),
                )

        nc.sync.dma_start(out=wt_sb[:, 22 * 128:], in_=wts[:, 22 * 128:])
        bias_sb = const_pool.tile([128, MCH], F32)
        nc.sync.dma_start(out=bias_sb[:], in_=bias[:, :])

        # xhat: frequency-basis transform of the whole padded image (the
        # borders of x are zero, so xhat borders transform to zero too).
        xhat = []
        for c in range(MCH):
            xh_t = xh_pool.tile([128, HP * HP], F32R, tag=f"xh{c}")
            xhat.append(xh_t)
        # padded-row ranges per fwd tile: 7 tiles of 8 rows + 1 of 2 rows
        fwd_rows = [(it * RPT, min(HP, (it + 1) * RPT)) for it in range(8)]

        def fwd_tile(it):
            """Transform padded pixel rows [r0, r1) of the image."""
            r0, r1 = fwd_rows[it]
            npx = (r1 - r0) * HP
            for c in range(MCH):
                ps = psf_pool.tile([128, RPT * HP], F32, tag="psf")
                for i in range(MCH):
                    rhs = xq[i][:, r0 * HP: r1 * HP]
                    nc.tensor.matmul(
                        ps[:, :npx], lhsT=blk(FWD_BLK(i, c)), rhs=rhs,
                        start=(i == 0), stop=(i == MCH - 1),
                    )
                nc.vector.tensor_copy(
                    xhat[c][:, r0 * HP: r1 * HP], ps[:, :npx]
                )

        # out viewed as [p(128), m(2), pix]: c = m*128 + p
        out_v = out.rearrange("(m p) h w -> p m (h w)", m=MCH)

        def conv_inv_tile(nt, ob):
            """Freq-domain conv + inverse transform for output tile nt."""
            ohat = []
            for c in range(MCH):
                ps = psc_pool.tile([128, NT], F32, tag="psc")
                n_mm = 0
                for kh in range(KK):
                    for kw in range(KK):
                        pos = kh * KK + kw
                        xhv = xhat[c][:].rearrange("p (h w) -> p h w", h=HP)
                        rhs = xhv[
                            :, nt * RPT + kh: nt * RPT + kh + RPT, kw: kw + W
                        ]
                        nc.tensor.matmul(
                            ps[:], lhsT=blk(CONV_BLK(pos, c)), rhs=rhs,
                            start=(n_mm == 0), stop=(n_mm == KK * KK - 1),
                        )
                        n_mm += 1
                oh = oh_pool.tile([128, NT], F32R, tag="oh")
                nc.vector.tensor_copy(oh[:], ps[:])
                ohat.append(oh)
            for m in range(MCH):
                ps = psi_pool.tile([128, NT], F32, tag="psi")
                for k in range(MCH):
                    nc.tensor.matmul(
                        ps[:], lhsT=blk(INV_BLK(k, m)), rhs=ohat[k][:],
                        start=(k == 0), stop=(k == MCH - 1),
                    )
                nc.scalar.activation(
                    ob[:, m, :], ps[:],
                    mybir.ActivationFunctionType.Identity,
                    bias=bias_sb[:, m: m + 1],
                )
                if nt == N_TILES - 1:
                    # tail: ship each chunk as soon as its bias-add lands
                    dma_eng = nc.scalar if m == 0 else nc.sync
                    dma_eng.dma_start(
                        out=out_v[:, m, ts(nt, NT)], in_=ob[:, m, :]
                    )

        def emit_out(nt, ob):
            if nt == N_TILES - 1:
                return  # shipped per-chunk inside conv_inv_tile
            # issue from ACT (it produced ob, so ordering is natural and
            # the issue overlaps the MM stream on other engines)
            nc.scalar.dma_start(out=out_v[:, :, ts(nt, NT)], in_=ob[:])

        # Interleave: fwd runs a few tiles ahead of conv (conv tile nt
        # reads padded xhat rows [nt*8, nt*8+9] = fwd tiles nt and nt+1);
        # the deep lead keeps the PE fed while conv weights stream in.
        for it in range(4):
            fwd_tile(it)
        for nt in range(N_TILES):
            if nt + 4 < len(fwd_rows):
                fwd_tile(nt + 4)
            ob = ob_pool.tile([128, MCH, NT], F32, tag="ob")
            conv_inv_tile(nt, ob)
            emit_out(nt, ob)


def _build_nc():
    nc = bacc.Bacc("TRN2", target_bir_lowering=False, debug=False)
    x = nc.dram_tensor("x", [C, HP, HP], F32R, kind="ExternalInput").ap()
    wts = nc.dram_tensor("wts", [128, N_BLKS * 128], F32R,
                         kind="ExternalInput").ap()
    bias = nc.dram_tensor("bias", [128, MCH], F32, kind="ExternalInput").ap()
    out = nc.dram_tensor("out", [C, H, W], F32, kind="ExternalOutput").ap()
    with tile.TileContext(nc) as tc:
        _kernel_body(tc, x, wts, bias, out)
    nc.compile()
    return nc


def kernel(x: np.ndarray, w: np.ndarray, b: np.ndarray) -> np.ndarray:
    global LAST_RESULT
    xp = np.pad(np.asarray(x, np.float32), ((0, 0), (0, 0), (1, 1), (1, 1)))
    xp = np.ascontiguousarray(xp)
    wts = _pack_weights(np.asarray(w, np.float32))
    b = np.ascontiguousarray(np.asarray(b, np.float32).reshape(MCH, 128).T)

    nc = _build_nc()
    in_maps = [{"x": xp[i], "wts": wts, "bias": b} for i in range(N_CORES)]
    trace = bool(int(os.environ.get("KERNEL_PROFILE", "0")))
    res = None
    last_err = None
    for attempt in range(3):
        try:
            res = run_bass_kernel_spmd(
                nc,
                in_maps,
                core_ids=list(range(N_CORES)),
                trace=trace,
            )
            break
        except Exception as e:  # transient device wedge -> retry
            last_err = e
    if res is None:
        raise last_err
    LAST_RESULT = res
    return np.stack([res.results[i]["out"] for i in range(N_CORES)], axis=0)



# revision 3
# speedup vs baseline: 1.3152x; 1.3152x over previous
"""BCM_Conv2d_fft kernel for Trainium2 (8 NeuronCores, batch-parallel).

The reference is a block-circulant 3x3 conv computed via per-block
rfft/irfft over the channel-block axis (block size 8). Per-frequency the
block products are independent, so in a real-DFT channel basis the
256->256 channel mixing matrix of each conv tap is block-diagonal with
frequency groups {f0:32, f4:32, f1:64} -> chunk0 and {f2:64, f3:64} ->
chunk1: the conv needs 9 matmuls per output tile per chunk (18 total),
which meets the K-streaming lower bound (9 positions x 256 components /
128 K-rows per pass).

The DFT (fwd) and inverse (inv) stages exploit a finer structure: each
channel block's components come only from its own 8 channels, so the
A / Ainv 128x128 chunk blocks have only 64 useful rows / cols. With a
partition layout that groups blocks 0-15 and 16-31 into 64-partition
halves (chunk1 flipped), fwd runs as 2 passes (one per input chunk,
producing halves of BOTH xhat chunks) and inv as 2 passes (one per
output chunk, consuming assembled P/Q tiles) - all inter-stage
PSUM->SBUF copies stay partition-aligned. Per output tile: 2 fwd + 18
conv + 2 inv passes (was 4 + 18 + 4).

Everything streams in bf16 (PSUM accumulates f32): same PE rate as
float32r at these tile sizes, but half the HBM traffic for x, weights
and out, and half-rate LDWEIGHTS via FWL. Max rel err ~3e-3 (tol 2e-2).

Sharding: batch B=8 -> one image per core.
"""

import os

import ml_dtypes
import numpy as np

import concourse.bacc as bacc
import concourse.mybir as mybir
import concourse.tile as tile
from concourse.bass import ts
from concourse.bass_utils import run_bass_kernel_spmd

N_CORES = 8
C = 256
H = W = 56
HP = H + 2
KK = 3
BS = 8
L = H * W
RPT = 8                  # output rows per tile
NT = RPT * W             # 448 pixels per tile
N_TILES = L // NT        # 7
MCH = C // 128           # 2 channel chunks

F32 = mybir.dt.float32
BF16 = mybir.dt.bfloat16
NP_BF16 = ml_dtypes.bfloat16

# weight block column indices in the packed wts tensor [128, 22*128]
FWD_BLK = lambda i: i                    # i = input chunk
CONV_BLK = lambda pos, c: 2 + pos * MCH + c
INV_BLK = lambda m: 20 + m               # m = output chunk
N_BLKS = 22

# real-DFT components per chunk: chunk0 = {f0, f1re, f1im, f4},
# chunk1 = {f2re, f2im, f3re, f3im} (closed under conv's re/im mixing)
C0 = [0, 1, 2, 7]
C1 = [3, 4, 5, 6]

LAST_RESULT = None


def _pc(c, bk, j):
    """Partition of (block bk, comp-index j) within xhat chunk c.

    chunk0: blocks 0-15 at parts 0-63; chunk1: blocks 16-31 at parts
    0-63 (flipped so all fwd/inv half-copies are partition-aligned).
    """
    if c == 0:
        return (bk % 16) * 4 + j + 64 * (bk // 16)
    return (bk % 16) * 4 + j + 64 * (1 - bk // 16)


def _pack_weights(w: np.ndarray) -> np.ndarray:
    """-> [128, 22*128] bf16: lhsT blocks for fwd(2), conv(18), inv(2)."""
    F = np.zeros((8, 8))
    FI = np.fft.rfft(np.eye(8), axis=-1)
    F[0] = FI[:, 0].real
    F[1], F[2] = FI[:, 1].real, FI[:, 1].imag
    F[3], F[4] = FI[:, 2].real, FI[:, 2].imag
    F[5], F[6] = FI[:, 3].real, FI[:, 3].imag
    F[7] = FI[:, 4].real
    Finv = np.linalg.inv(F)
    wf = np.fft.rfft(w.astype(np.float64), axis=-1)  # [32, 288, 5]

    wts = np.zeros((128, N_BLKS * 128), np.float64)

    def put(idx, lhsT):
        wts[:, idx * 128:(idx + 1) * 128] = lhsT

    # fwd pass i (K = x chunk i = blocks 16i..16i+15): M low half feeds
    # chunk i's parts 0-63, high half feeds the other chunk's parts
    # 64-127 (both hold blocks 16i..16i+15 by the _pc layout).
    for i in range(MCH):
        Lk = np.zeros((128, 128))
        own, other = (C0, C1) if i == 0 else (C1, C0)
        for bkl in range(16):
            for j, comp in enumerate(own):
                Lk[bkl * 8:(bkl + 1) * 8, bkl * 4 + j] = F[comp]
            for j, comp in enumerate(other):
                Lk[bkl * 8:(bkl + 1) * 8, 64 + bkl * 4 + j] = F[comp]
        put(FWD_BLK(i), Lk)

    for pos in range(9):
        for c in range(MCH):
            M = np.zeros((128, 128))
            for pb in range(32):
                rp = lambda j: _pc(c, pb, j)
                for kb in range(32):
                    cp = lambda j: _pc(c, kb, j)
                    Wc = wf[pb, pos * 32 + kb, :]
                    if c == 0:
                        M[rp(0), cp(0)] += Wc[0].real            # f0
                        M[rp(3), cp(3)] += Wc[4].real            # f4
                        Wr, Wi = Wc[1].real, Wc[1].imag          # f1
                        M[rp(1), cp(1)] += Wr
                        M[rp(1), cp(2)] += -Wi
                        M[rp(2), cp(1)] += Wi
                        M[rp(2), cp(2)] += Wr
                    else:
                        Wr, Wi = Wc[2].real, Wc[2].imag          # f2
                        M[rp(0), cp(0)] += Wr
                        M[rp(0), cp(1)] += -Wi
                        M[rp(1), cp(0)] += Wi
                        M[rp(1), cp(1)] += Wr
                        Wr, Wi = Wc[3].real, Wc[3].imag          # f3
                        M[rp(2), cp(2)] += Wr
                        M[rp(2), cp(3)] += -Wi
                        M[rp(3), cp(2)] += Wi
                        M[rp(3), cp(3)] += Wr
            put(CONV_BLK(pos, c), M.T)

    # inv pass m consumes P (m=0) / Q (m=1): parts 0-63 hold this out
    # chunk's blocks from its own-chunk conv psum, 64-127 from the other
    for mc in range(MCH):
        Lk = np.zeros((128, 128))
        for k in range(128):
            half, kk = k // 64, k % 64
            bkl, j = kk // 4, kk % 4
            comp = (C0 if (half == 0) == (mc == 0) else C1)[j]
            for e in range(8):
                Lk[k, bkl * 8 + e] = Finv[e, comp]
        put(INV_BLK(mc), Lk)
    return wts.astype(NP_BF16)


def _kernel_body(tc, x, wts, bias, out):
    nc = tc.nc
    ident = mybir.ActivationFunctionType.Identity
    with (
        tc.tile_pool(name="const", bufs=1) as const_pool,
        tc.tile_pool(name="xp", bufs=1) as xp_pool,
        tc.tile_pool(name="xh", bufs=1) as xh_pool,
        tc.tile_pool(name="oh", bufs=6) as oh_pool,
        tc.tile_pool(name="ob", bufs=4) as ob_pool,
        tc.tile_pool(name="psf", bufs=3, space="PSUM") as psf_pool,
        tc.tile_pool(name="psc", bufs=3, space="PSUM") as psc_pool,
        tc.tile_pool(name="psi", bufs=2, space="PSUM") as psi_pool,
    ):
        # DMA issue order is first-needed-first: fwd weights, then the
        # first input rows; conv weights stream on the ACT queue in
        # parallel with the remaining x rows on sync/gpsimd queues.
        wt_sb = const_pool.tile([128, N_BLKS * 128], BF16)
        blk = lambda idx: wt_sb[:, ts(idx, 128)]
        nc.sync.dma_start(out=wt_sb[:, 0:2 * 128], in_=wts[:, 0:2 * 128])

        xq = []
        for i in range(MCH):
            xq_t = xp_pool.tile([128, HP * HP], BF16, tag=f"xp{i}")
            xq.append(xq_t)
        row_splits = [0, 8, 16, 34, HP]
        eng = [nc.sync, nc.gpsimd]
        for r0, r1 in zip(row_splits[:-1], row_splits[1:]):
            for i in range(MCH):
                eng[i].dma_start(
                    out=xq[i][:, r0 * HP:r1 * HP],
                    in_=x[ts(i, 128), r0:r1, :].rearrange("p h w -> p (h w)"),
                )
            if r0 == 0:
                nc.scalar.dma_start(out=wt_sb[:, 2 * 128:11 * 128],
                                    in_=wts[:, 2 * 128:11 * 128])
                nc.scalar.dma_start(out=wt_sb[:, 11 * 128:20 * 128],
                                    in_=wts[:, 11 * 128:20 * 128])
            if r0 == 16:
                nc.sync.dma_start(out=wt_sb[:, 20 * 128:],
                                  in_=wts[:, 20 * 128:])
        bias_sb = const_pool.tile([128, MCH], F32)
        nc.sync.dma_start(out=bias_sb[:], in_=bias[:, :])

        # xhat: frequency-basis transform of the whole padded image (the
        # borders of x are zero, so xhat borders transform to zero too).
        xhat = []
        for c in range(MCH):
            xh_t = xh_pool.tile([128, HP * HP], BF16, tag=f"xh{c}")
            xhat.append(xh_t)
        # padded-row ranges per fwd tile: 7 tiles of 8 rows + 1 of 2 rows
        fwd_rows = [(it * RPT, min(HP, (it + 1) * RPT)) for it in range(8)]

        def fwd_tile(it):
            """Transform padded pixel rows [r0, r1) of the image."""
            r0, r1 = fwd_rows[it]
            npx = (r1 - r0) * HP
            sl = slice(r0 * HP, r1 * HP)
            ps = []
            for i in range(MCH):
                p = psf_pool.tile([128, RPT * HP], F32, tag="psf")
                nc.tensor.matmul(p[:, :npx], lhsT=blk(FWD_BLK(i)),
                                 rhs=xq[i][:, sl], start=True, stop=True)
                ps.append(p)
            # psA = [xh0 lo | xh1 hi], psB = [xh1 lo | xh0 hi]; all four
            # copies are partition-aligned. psA on vector, psB on scalar
            # so the two PSUM banks drain in parallel.
            nc.vector.tensor_copy(xhat[0][0:64, sl], ps[0][0:64, :npx])
            nc.vector.tensor_copy(xhat[1][64:128, sl], ps[0][64:128, :npx])
            nc.scalar.activation(xhat[1][0:64, sl], ps[1][0:64, :npx], ident)
            nc.scalar.activation(xhat[0][64:128, sl], ps[1][64:128, :npx],
                                 ident)

        # out viewed as [p(128), m(2), pix]: c = m*128 + p
        out_v = out.rearrange("(m p) h w -> p m (h w)", m=MCH)

        def conv_tile(nt):
            """Freq-domain conv for output tile nt -> assembled P, Q."""
            pscs = []
            for c in range(MCH):
                psum = psc_pool.tile([128, NT], F32, tag="psc")
                n_mm = 0
                xhv = xhat[c][:].rearrange("p (h w) -> p h w", h=HP)
                for kh in range(KK):
                    for kw in range(KK):
                        pos = kh * KK + kw
                        rhs = xhv[
                            :, nt * RPT + kh: nt * RPT + kh + RPT, kw: kw + W
                        ]
                        nc.tensor.matmul(
                            psum[:], lhsT=blk(CONV_BLK(pos, c)), rhs=rhs,
                            start=(n_mm == 0), stop=(n_mm == KK * KK - 1),
                        )
                        n_mm += 1
                pscs.append(psum)
            # P/Q feed inv passes for out chunk 0/1; partition-aligned
            # half-copies, vector on psc0's bank, scalar on psc1's.
            P = oh_pool.tile([128, NT], BF16, tag="oh")
            Q = oh_pool.tile([128, NT], BF16, tag="oh")
            nc.vector.tensor_copy(P[0:64, :], pscs[0][0:64, :])
            nc.vector.tensor_copy(Q[64:128, :], pscs[0][64:128, :])
            nc.scalar.activation(Q[0:64, :], pscs[1][0:64, :], ident)
            nc.scalar.activation(P[64:128, :], pscs[1][64:128, :], ident)
            return [P, Q]

        def inv_tile(nt, pq, ob):
            """Inverse transform + bias for output tile nt, ship it."""
            for m in range(MCH):
                psum = psi_pool.tile([128, NT], F32, tag="psi")
                nc.tensor.matmul(psum[:], lhsT=blk(INV_BLK(m)), rhs=pq[m][:],
                                 start=True, stop=True)
                if m == 0:
                    nc.vector.tensor_scalar_add(ob[:, m, :], psum[:],
                                                bias_sb[:, m:m + 1])
                else:
                    nc.scalar.activation(ob[:, m, :], psum[:], ident,
                                         bias=bias_sb[:, m:m + 1])
                if nt == N_TILES - 1:
                    # tail: ship each chunk as soon as its bias-add lands
                    dma_eng = nc.gpsimd if m == 0 else nc.sync
                    dma_eng.dma_start(
                        out=out_v[:, m, ts(nt, NT)], in_=ob[:, m, :]
                    )
            if nt < N_TILES - 1:
                nc.gpsimd.dma_start(out=out_v[:, :, ts(nt, NT)], in_=ob[:])

        # Interleave: fwd runs a few tiles ahead of conv (conv tile nt
        # reads padded xhat rows [nt*8, nt*8+9] = fwd tiles nt and nt+1);
        # inv for tile nt is issued after conv tile nt+1 so the P/Q
        # copies complete in the shadow of the next conv's matmuls.
        for it in range(4):
            fwd_tile(it)
        pending = None
        for nt in range(N_TILES):
            if nt + 4 < len(fwd_rows):
                fwd_tile(nt + 4)
            pq = conv_tile(nt)
            if pending is not None:
                inv_tile(*pending)
            ob = ob_pool.tile([128, MCH, NT], BF16, tag="ob")
            pending = (nt, pq, ob)
        inv_tile(*pending)


def _build_nc():
    nc = bacc.Bacc("TRN2", target_bir_lowering=False, debug=False)
    x = nc.dram_tensor("x", [C, HP, HP], BF16, kind="ExternalInput").ap()
    wts = nc.dram_tensor("wts", [128, N_BLKS * 128], BF16,
                         kind="ExternalInput").ap()
    bias = nc.dram_tensor("bias", [128, MCH], F32, kind="ExternalInput").ap()
    out = nc.dram_tensor("out", [C, H, W], BF16, kind="ExternalOutput").ap()
    with tile.TileContext(nc) as tc:
        _kernel_body(tc, x, wts, bias, out)
    nc.compile()
    return nc


def kernel(x: np.ndarray, w: np.ndarray, b: np.ndarray) -> np.ndarray:
    global LAST_RESULT
    xp = np.pad(np.asarray(x, np.float32), ((0, 0), (0, 0), (1, 1), (1, 1)))
    xp = np.ascontiguousarray(xp).astype(NP_BF16)
    wts = _pack_weights(np.asarray(w, np.float32))
    b = np.ascontiguousarray(np.asarray(b, np.float32).reshape(MCH, 128).T)

    nc = _build_nc()
    in_maps = [{"x": xp[i], "wts": wts, "bias": b} for i in range(N_CORES)]
    trace = bool(int(os.environ.get("KERNEL_PROFILE", "0")))
    res = None
    last_err = None
    for attempt in range(3):
        try:
            res = run_bass_kernel_spmd(
                nc,
                in_maps,
                core_ids=list(range(N_CORES)),
                trace=trace,
            )
            break
        except Exception as e:  # transient device wedge -> retry
            last_err = e
    if res is None:
        raise last_err
    LAST_RESULT = res
    return np.stack(
        [res.results[i]["out"] for i in range(N_CORES)], axis=0
    ).astype(np.float32)


# revision 6
# speedup vs baseline: 1.3479x; 1.0249x over previous
"""BCM_Conv2d_fft kernel for Trainium2 (8 NeuronCores, batch-parallel).

The reference is a block-circulant 3x3 conv computed via per-block
rfft/irfft over the channel-block axis (block size 8). Per-frequency the
block products are independent, so in a real-DFT channel basis the
256->256 channel mixing matrix of each conv tap is block-diagonal with
frequency groups {f0:32, f4:32, f1:64} -> chunk0 and {f2:64, f3:64} ->
chunk1: the conv needs 9 matmuls per output tile per chunk (18 total),
which meets the K-streaming lower bound (9 positions x 256 components /
128 K-rows per pass).

The DFT (fwd) and inverse (inv) stages exploit a finer structure: each
channel block's components come only from its own 8 channels, so the
A / Ainv 128x128 chunk blocks have only 64 useful rows / cols. With a
partition layout that groups blocks 0-15 and 16-31 into 64-partition
halves (chunk1 flipped), fwd runs as 2 passes (one per input chunk,
producing halves of BOTH xhat chunks) and inv as 2 passes (one per
output chunk, consuming assembled P/Q tiles) - all inter-stage
PSUM->SBUF copies stay partition-aligned. Per output tile: 2 fwd + 18
conv + 2 inv passes (was 4 + 18 + 4).

Everything streams in bf16 (PSUM accumulates f32): same PE rate as
float32r at these tile sizes, but half the HBM traffic for x, weights
and out, and half-rate LDWEIGHTS via FWL. Max rel err ~3e-3 (tol 2e-2).

Sharding: batch B=8 -> one image per core.
"""

import os

import ml_dtypes
import numpy as np

import concourse.bacc as bacc
import concourse.mybir as mybir
import concourse.tile as tile
from concourse.bass import ts
from concourse.bass_utils import run_bass_kernel_spmd

N_CORES = 8
C = 256
H = W = 56
HP = H + 2
KK = 3
BS = 8
L = H * W
RPT = 8                  # output rows per tile
NT = RPT * W             # 448 pixels per tile
N_TILES = L // NT        # 7
MCH = C // 128           # 2 channel chunks

F32 = mybir.dt.float32
BF16 = mybir.dt.bfloat16
NP_BF16 = ml_dtypes.bfloat16

# weight block column indices in the packed wts tensor [128, 22*128]
FWD_BLK = lambda i: i                    # i = input chunk
CONV_BLK = lambda pos, c: 2 + pos * MCH + c
INV_BLK = lambda m: 20 + m               # m = output chunk
N_BLKS = 22

# real-DFT components per chunk: chunk0 = {f0, f1re, f1im, f4},
# chunk1 = {f2re, f2im, f3re, f3im} (closed under conv's re/im mixing)
C0 = [0, 1, 2, 7]
C1 = [3, 4, 5, 6]

LAST_RESULT = None


def _pc(c, bk, j):
    """Partition of (block bk, comp-index j) within xhat chunk c.

    chunk0: blocks 0-15 at parts 0-63; chunk1: blocks 16-31 at parts
    0-63 (flipped so all fwd/inv half-copies are partition-aligned).
    """
    if c == 0:
        return (bk % 16) * 4 + j + 64 * (bk // 16)
    return (bk % 16) * 4 + j + 64 * (1 - bk // 16)


def _pack_weights(w: np.ndarray) -> np.ndarray:
    """-> [128, 22*128] bf16: lhsT blocks for fwd(2), conv(18), inv(2)."""
    F = np.zeros((8, 8))
    FI = np.fft.rfft(np.eye(8), axis=-1)
    F[0] = FI[:, 0].real
    F[1], F[2] = FI[:, 1].real, FI[:, 1].imag
    F[3], F[4] = FI[:, 2].real, FI[:, 2].imag
    F[5], F[6] = FI[:, 3].real, FI[:, 3].imag
    F[7] = FI[:, 4].real
    Finv = np.linalg.inv(F)
    wf = np.fft.rfft(w.astype(np.float64), axis=-1)  # [32, 288, 5]

    wts = np.zeros((128, N_BLKS * 128), np.float64)

    def put(idx, lhsT):
        wts[:, idx * 128:(idx + 1) * 128] = lhsT

    # fwd pass i (K = x chunk i = blocks 16i..16i+15): M low half feeds
    # chunk i's parts 0-63, high half feeds the other chunk's parts
    # 64-127 (both hold blocks 16i..16i+15 by the _pc layout).
    for i in range(MCH):
        Lk = np.zeros((128, 128))
        own, other = (C0, C1) if i == 0 else (C1, C0)
        for bkl in range(16):
            for j, comp in enumerate(own):
                Lk[bkl * 8:(bkl + 1) * 8, bkl * 4 + j] = F[comp]
            for j, comp in enumerate(other):
                Lk[bkl * 8:(bkl + 1) * 8, 64 + bkl * 4 + j] = F[comp]
        put(FWD_BLK(i), Lk)

    for pos in range(9):
        for c in range(MCH):
            M = np.zeros((128, 128))
            for pb in range(32):
                rp = lambda j: _pc(c, pb, j)
                for kb in range(32):
                    cp = lambda j: _pc(c, kb, j)
                    Wc = wf[pb, pos * 32 + kb, :]
                    if c == 0:
                        M[rp(0), cp(0)] += Wc[0].real            # f0
                        M[rp(3), cp(3)] += Wc[4].real            # f4
                        Wr, Wi = Wc[1].real, Wc[1].imag          # f1
                        M[rp(1), cp(1)] += Wr
                        M[rp(1), cp(2)] += -Wi
                        M[rp(2), cp(1)] += Wi
                        M[rp(2), cp(2)] += Wr
                    else:
                        Wr, Wi = Wc[2].real, Wc[2].imag          # f2
                        M[rp(0), cp(0)] += Wr
                        M[rp(0), cp(1)] += -Wi
                        M[rp(1), cp(0)] += Wi
                        M[rp(1), cp(1)] += Wr
                        Wr, Wi = Wc[3].real, Wc[3].imag          # f3
                        M[rp(2), cp(2)] += Wr
                        M[rp(2), cp(3)] += -Wi
                        M[rp(3), cp(2)] += Wi
                        M[rp(3), cp(3)] += Wr
            put(CONV_BLK(pos, c), M.T)

    # inv pass m consumes P (m=0) / Q (m=1): parts 0-63 hold this out
    # chunk's blocks from its own-chunk conv psum, 64-127 from the other
    for mc in range(MCH):
        Lk = np.zeros((128, 128))
        for k in range(128):
            half, kk = k // 64, k % 64
            bkl, j = kk // 4, kk % 4
            comp = (C0 if (half == 0) == (mc == 0) else C1)[j]
            for e in range(8):
                Lk[k, bkl * 8 + e] = Finv[e, comp]
        put(INV_BLK(mc), Lk)
    return wts.astype(NP_BF16)


def _kernel_body(tc, x, wts, bias, out):
    nc = tc.nc
    ident = mybir.ActivationFunctionType.Identity
    with (
        tc.tile_pool(name="const", bufs=1) as const_pool,
        tc.tile_pool(name="xp", bufs=1) as xp_pool,
        tc.tile_pool(name="xh", bufs=1) as xh_pool,
        tc.tile_pool(name="oh", bufs=6) as oh_pool,
        tc.tile_pool(name="ob", bufs=4) as ob_pool,
        tc.tile_pool(name="psf", bufs=3, space="PSUM") as psf_pool,
        tc.tile_pool(name="psc", bufs=3, space="PSUM") as psc_pool,
        tc.tile_pool(name="psi", bufs=2, space="PSUM") as psi_pool,
    ):
        # DMA issue order is first-needed-first: fwd weights, then the
        # first input rows; conv weights stream on the ACT queue in
        # parallel with the remaining x rows on sync/gpsimd queues.
        wt_sb = const_pool.tile([128, N_BLKS * 128], BF16)
        blk = lambda idx: wt_sb[:, ts(idx, 128)]

        xq = []
        for i in range(MCH):
            xq_t = xp_pool.tile([128, HP * HP], BF16, tag=f"xp{i}")
            xq.append(xq_t)
        row_splits = [0, 8, 16, 34, HP]
        eng = [nc.sync, nc.gpsimd]
        for r0, r1 in zip(row_splits[:-1], row_splits[1:]):
            for i in range(MCH):
                eng[i].dma_start(
                    out=xq[i][:, r0 * HP:r1 * HP],
                    in_=x[ts(i, 128), r0:r1, :].rearrange("p h w -> p (h w)"),
                )
            if r0 == 0:
                # fwd weights lead the ACT queue so the first matmul is
                # gated only by the first x rows; conv weights follow
                nc.scalar.dma_start(out=wt_sb[:, 0:2 * 128],
                                    in_=wts[:, 0:2 * 128])
                nc.scalar.dma_start(out=wt_sb[:, 2 * 128:11 * 128],
                                    in_=wts[:, 2 * 128:11 * 128])
                nc.scalar.dma_start(out=wt_sb[:, 11 * 128:20 * 128],
                                    in_=wts[:, 11 * 128:20 * 128])
            if r0 == 8:
                nc.sync.dma_start(out=wt_sb[:, 20 * 128:],
                                  in_=wts[:, 20 * 128:])
        bias_sb = const_pool.tile([128, MCH], F32)
        nc.sync.dma_start(out=bias_sb[:], in_=bias[:, :])

        # xhat: frequency-basis transform of the whole padded image (the
        # borders of x are zero, so xhat borders transform to zero too).
        xhat = []
        for c in range(MCH):
            xh_t = xh_pool.tile([128, HP * HP], BF16, tag=f"xh{c}")
            xhat.append(xh_t)
        # padded-row ranges per fwd tile: 7 tiles of 8 rows + 1 of 2 rows
        fwd_rows = [(it * RPT, min(HP, (it + 1) * RPT)) for it in range(8)]

        def fwd_tile(it):
            """Transform padded pixel rows [r0, r1) of the image."""
            r0, r1 = fwd_rows[it]
            npx = (r1 - r0) * HP
            sl = slice(r0 * HP, r1 * HP)
            ps = []
            for i in range(MCH):
                p = psf_pool.tile([128, RPT * HP], F32, tag="psf")
                nc.tensor.matmul(p[:, :npx], lhsT=blk(FWD_BLK(i)),
                                 rhs=xq[i][:, sl], start=True, stop=True)
                ps.append(p)
            # psA = [xh0 lo | xh1 hi], psB = [xh1 lo | xh0 hi]; all four
            # copies are partition-aligned. psA on vector, psB on scalar
            # so the two PSUM banks drain in parallel, and the chunk0
            # halves drain first on each engine (conv does chunk0 first).
            nc.vector.tensor_copy(xhat[0][0:64, sl], ps[0][0:64, :npx])
            nc.vector.tensor_copy(xhat[1][64:128, sl], ps[0][64:128, :npx])
            nc.scalar.activation(xhat[0][64:128, sl], ps[1][64:128, :npx],
                                 ident)
            nc.scalar.activation(xhat[1][0:64, sl], ps[1][0:64, :npx], ident)

        # out viewed as [p(128), m(2), pix]: c = m*128 + p
        out_v = out.rearrange("(m p) h w -> p m (h w)", m=MCH)

        def conv_tile(nt):
            """Freq-domain conv for output tile nt -> assembled P, Q."""
            pscs = []
            for c in range(MCH):
                psum = psc_pool.tile([128, NT], F32, tag="psc")
                n_mm = 0
                xhv = xhat[c][:].rearrange("p (h w) -> p h w", h=HP)
                for kh in range(KK):
                    for kw in range(KK):
                        pos = kh * KK + kw
                        rhs = xhv[
                            :, nt * RPT + kh: nt * RPT + kh + RPT, kw: kw + W
                        ]
                        nc.tensor.matmul(
                            psum[:], lhsT=blk(CONV_BLK(pos, c)), rhs=rhs,
                            start=(n_mm == 0), stop=(n_mm == KK * KK - 1),
                        )
                        n_mm += 1
                pscs.append(psum)
            # P/Q feed inv passes for out chunk 0/1; partition-aligned
            # half-copies, vector on psc0's bank, scalar on psc1's.
            P = oh_pool.tile([128, NT], BF16, tag="oh")
            Q = oh_pool.tile([128, NT], BF16, tag="oh")
            nc.vector.tensor_copy(P[0:64, :], pscs[0][0:64, :])
            nc.vector.tensor_copy(Q[64:128, :], pscs[0][64:128, :])
            nc.scalar.activation(Q[0:64, :], pscs[1][0:64, :], ident)
            nc.scalar.activation(P[64:128, :], pscs[1][64:128, :], ident)
            return [P, Q]

        def inv_tile(nt, pq, ob):
            """Inverse transform + bias for output tile nt, ship it."""
            for m in range(MCH):
                psum = psi_pool.tile([128, NT], F32, tag="psi")
                nc.tensor.matmul(psum[:], lhsT=blk(INV_BLK(m)), rhs=pq[m][:],
                                 start=True, stop=True)
                if m == 0:
                    nc.vector.tensor_scalar_add(ob[:, m, :], psum[:],
                                                bias_sb[:, m:m + 1])
                else:
                    nc.scalar.activation(ob[:, m, :], psum[:], ident,
                                         bias=bias_sb[:, m:m + 1])
                if nt == N_TILES - 1:
                    # tail: ship each chunk as soon as its bias-add lands
                    dma_eng = nc.gpsimd if m == 0 else nc.sync
                    dma_eng.dma_start(
                        out=out_v[:, m, ts(nt, NT)], in_=ob[:, m, :]
                    )
            if nt < N_TILES - 1:
                nc.gpsimd.dma_start(out=out_v[:, :, ts(nt, NT)], in_=ob[:])

        # Interleave: conv tile nt reads padded xhat rows [nt*8, nt*8+9]
        # = fwd tiles nt and nt+1, so fwd leads conv by two tiles (a
        # deeper lead would pile fwd drains onto vector/scalar while the
        # PE idles); inv for tile nt is issued after conv tile nt+1 so
        # the P/Q copies complete in the shadow of the next conv.
        fwd_tile(0)
        fwd_tile(1)
        pending = None
        for nt in range(N_TILES):
            pq = conv_tile(nt)
            if nt + 2 < len(fwd_rows):
                fwd_tile(nt + 2)
            if pending is not None:
                inv_tile(*pending)
            ob = ob_pool.tile([128, MCH, NT], BF16, tag="ob")
            pending = (nt, pq, ob)
        inv_tile(*pending)


def _build_nc():
    nc = bacc.Bacc("TRN2", target_bir_lowering=False, debug=False)
    x = nc.dram_tensor("x", [C, HP, HP], BF16, kind="ExternalInput").ap()
    wts = nc.dram_tensor("wts", [128, N_BLKS * 128], BF16,
                         kind="ExternalInput").ap()
    bias = nc.dram_tensor("bias", [128, MCH], F32, kind="ExternalInput").ap()
    out = nc.dram_tensor("out", [C, H, W], BF16, kind="ExternalOutput").ap()
    with tile.TileContext(nc) as tc:
        _kernel_body(tc, x, wts, bias, out)
    nc.compile()
    return nc


def kernel(x: np.ndarray, w: np.ndarray, b: np.ndarray) -> np.ndarray:
    global LAST_RESULT
    xp = np.pad(np.asarray(x, np.float32), ((0, 0), (0, 0), (1, 1), (1, 1)))
    xp = np.ascontiguousarray(xp).astype(NP_BF16)
    wts = _pack_weights(np.asarray(w, np.float32))
    b = np.ascontiguousarray(np.asarray(b, np.float32).reshape(MCH, 128).T)

    nc = _build_nc()
    in_maps = [{"x": xp[i], "wts": wts, "bias": b} for i in range(N_CORES)]
    trace = bool(int(os.environ.get("KERNEL_PROFILE", "0")))
    res = None
    last_err = None
    for attempt in range(3):
        try:
            res = run_bass_kernel_spmd(
                nc,
                in_maps,
                core_ids=list(range(N_CORES)),
                trace=trace,
            )
            break
        except Exception as e:  # transient device wedge -> retry
            last_err = e
    if res is None:
        raise last_err
    LAST_RESULT = res
    return np.stack(
        [res.results[i]["out"] for i in range(N_CORES)], axis=0
    ).astype(np.float32)
